# revision 14
# baseline (speedup 1.0000x reference)
"""Trainium2 Bass kernel for nn_AttnOnlyTransformer_55929064128766.

Reference model: B=4, S=2048, D=2048 (=vocab), DQK=128, L=4 layers.
  enc0 = one_hot(token_ids, D) + sinusoidal_PE(S, D)
  per layer: q = enc@Wq; k = enc@Wk; A = softmax(mask(q k^T / sqrt(DQK)));
             enc = A @ enc
  output: enc  [B, S, D] f32

Sharding (8 cores): data-parallel over batch (4 pairs) x column-parallel
over D within each pair (Dc = 1024 columns of enc per core).  Per layer
each core computes partial q/k from its columns; a pairwise AllReduce
(groups [0,1],[2,3],[4,5],[6,7]) completes the projections; scores are
replicated within the pair; A @ enc splits cleanly by columns and the
column sharding of enc is preserved across layers.

All matmuls use float32r (f32 storage, ~1.5e-4 matmul relative error,
4x faster than plain f32 on the PE).
"""

import math

import numpy as np

B, S, D, DQK, L = 4, 2048, 2048, 128, 4
SPLIT = 2                 # cores per batch (column split factor)
DC = D // SPLIT           # columns of enc owned by one core
N_CORES = B * SPLIT
NT = S // 128             # number of 128-row tiles of the sequence (16)
NDT = DC // 128           # number of 128-col d-tiles per core (8)
SCALE = 1.0 / math.sqrt(DQK)
GROUPS = [[2 * i, 2 * i + 1] for i in range(B)]

_CACHED = {}


def _build(reps=1, skip_cc=False):
    import concourse.bass as bass  # noqa: F401
    import concourse.mybir as mybir
    import concourse.tile as tile
    from concourse import bacc

    F32 = mybir.dt.float32
    F32R = mybir.dt.float32r
    BF16 = mybir.dt.bfloat16
    Exp = mybir.ActivationFunctionType.Exp
    Copy = mybir.ActivationFunctionType.Copy

    nc = bacc.Bacc("TRN2", target_bir_lowering=False, debug=False,
                   num_devices=N_CORES)

    # ---- I/O ----
    pe_nat = nc.dram_tensor("pe_nat", [S, DC], F32R, kind="ExternalInput").ap()
    tok_col = nc.dram_tensor("tok_col", [128, NT], F32, kind="ExternalInput").ap()
    iota_nat = nc.dram_tensor("iota_nat", [128, DC], F32, kind="ExternalInput").ap()
    w_stk_in = nc.dram_tensor("w_stk", [L, 128, NDT * 256], F32R,
                              kind="ExternalInput").ap()
    ident_in = nc.dram_tensor("ident", [128, 128], F32R, kind="ExternalInput").ap()
    identb_in = nc.dram_tensor("identb", [128, 128], BF16, kind="ExternalInput").ap()
    utmask_in = nc.dram_tensor("utmask", [128, 128], F32, kind="ExternalInput").ap()
    ones_in = nc.dram_tensor("ones", [128, 2], F32R, kind="ExternalInput").ap()
    out_dram = nc.dram_tensor("out", [S, DC], F32R, kind="ExternalOutput").ap()

    # two half-allgathers per layer; half h carries q/k row blocks 8h..8h+7
    cc_in_h = [nc.dram_tensor(f"cc_in{h}", [128, S], BF16, kind="Internal").ap()
               for h in range(2)]
    cc_out_h = [nc.dram_tensor(f"cc_out{h}", [SPLIT, 128, S], BF16,
                               kind="Internal").ap()
                for h in range(2)]

    with tile.TileContext(nc) as tc:
        with (
            tc.tile_pool(name="state", bufs=1) as state,
            tc.tile_pool(name="consts", bufs=1) as consts,
            tc.tile_pool(name="wpool", bufs=2) as wpool,
        ):
            enc = [state.tile([128, DC], F32R, tag=f"enc{u}", name=f"enc{u}")
                   for u in range(NT)]
            qkT = state.tile([128, 2 * S], BF16, tag="qkT", name="qkT")
            carry = state.tile([128, 2 * S], BF16, tag="carry", name="carry")
            y_sb = state.tile([128, NT * 258], F32R, tag="y", name="y")
            qk_pe = state.tile([128, 2 * S], BF16, tag="qkpe", name="qk_pe")

            ident = consts.tile([128, 128], F32R, tag="ident")
            identb = consts.tile([128, 128], BF16, tag="identb")
            utmask = consts.tile([128, 128], F32, tag="utmask")
            ones = consts.tile([128, 2], F32R, tag="ones")
            nc.sync.dma_start(ident[:], ident_in)
            nc.sync.dma_start(identb[:], identb_in)
            nc.sync.dma_start(utmask[:], utmask_in)
            nc.sync.dma_start(ones[:], ones_in)

            for t in range(NT):
                nc.vector.tensor_copy(
                    y_sb[:, 258 * t + 256: 258 * t + 258], ones[:])

            def w_blk(w_tile, dt):
                return w_tile[:, dt * 256: dt * 256 + 256]

            def emit_ag_half(h):
                # carry cols [2048h, 2048h+2048) hold q/k row blocks 8h..8h+7
                sl = slice(S * h, S * h + S)
                nc.sync.dma_start(cc_in_h[h], carry[:, sl])
                nc.gpsimd.collective_compute(
                    "AllGather",
                    mybir.AluOpType.bypass,
                    replica_groups=GROUPS,
                    ins=[cc_in_h[h]],
                    outs=[cc_out_h[h]],
                )

            # ---- build enc0 = PE + one_hot(tokens) ----
            with tc.tile_pool(name="tmp0", bufs=4) as tmp0:
                tok = tmp0.tile([128, NT], F32, tag="tok", name="tok")
                iota = tmp0.tile([128, DC], F32, tag="iota", name="iota")
                nc.sync.dma_start(tok[:], tok_col)
                nc.sync.dma_start(iota[:], iota_nat)
                for u in reversed(range(NT)):
                    nc.sync.dma_start(enc[u][:], pe_nat[u * 128:(u + 1) * 128, :])
                for u in reversed(range(NT)):
                    oh = tmp0.tile([128, DC], F32, tag="oh", name=f"oh{u}")
                    nc.vector.tensor_scalar(
                        oh[:], iota[:], tok[:, u:u + 1], None,
                        mybir.AluOpType.is_equal,
                    )
                    nc.vector.tensor_tensor(
                        enc[u][:], enc[u][:], oh[:], mybir.AluOpType.add,
                    )

            # ---- layers ----
            for rep in range(reps):
              for l0 in range(L):
                l = rep * L + l0

                # == phase P ==
                if l0 == 0 and rep == 0:
                    w_cur = wpool.tile([128, NDT * 256], F32R, tag="w",
                                       name=f"wcur{l}")
                    nc.sync.dma_start(w_cur[:], w_stk_in[0])
                if l0 < L - 1:
                    w_next = wpool.tile([128, NDT * 256], F32R, tag="w",
                                        name=f"wnext{l}")
                    nc.sync.dma_start(w_next[:], w_stk_in[l0 + 1])

                with tc.tile_pool(name=f"encT{l}", bufs=1) as encT_pool:
                    encT = [encT_pool.tile([128, S], F32R, tag=f"encT{j}",
                                           name=f"encT{l}_{j}")
                            for j in range(NDT)]
                    with tc.tile_pool(name=f"trps{l}", bufs=2,
                                      space="PSUM") as trps:
                        for j in range(NDT):
                            for g in reversed(range(NT // 4)):
                                umin = g * 4
                                ptr = trps.tile([128, 512], F32R, tag="tr",
                                                name=f"tr{l}_{j}_{g}")
                                for m in range(4):
                                    u = umin + m
                                    nc.tensor.transpose(
                                        ptr[:, m * 128:(m + 1) * 128],
                                        enc[u][:, j * 128:(j + 1) * 128],
                                        ident[:],
                                    )
                                nc.vector.tensor_copy(
                                    encT[j][:, umin * 128: umin * 128 + 512],
                                    ptr[:],
                                )

                    with tc.tile_pool(name=f"yps{l}", bufs=2,
                                      space="PSUM") as yps:
                        # layer 0: qk partial = enc0 @ W0 (natural), to carry
                        if l0 == 0 and rep == 0:
                            for t in reversed(range(NT)):
                                py = yps.tile([128, 256], F32, tag="y0",
                                              name=f"py0_{l}_{t}")
                                for dt in range(NDT):
                                    nc.tensor.matmul(
                                        py[:],
                                        encT[dt][:, t * 128:(t + 1) * 128],
                                        w_blk(w_cur, dt),
                                        start=(dt == 0), stop=(dt == NDT - 1),
                                    )
                                nc.vector.tensor_copy(
                                    carry[:, 256 * t: 256 * t + 256], py[:])
                                if not skip_cc:
                                    if t == 8:
                                        emit_ag_half(1)
                                    elif t == 0:
                                        emit_ag_half(0)

                        # y_{l+1} = enc_l @ W_{l+1} (overlaps the allgather)
                        if l0 < L - 1:
                            for t in range(NT):
                                py = yps.tile([128, 256], F32, tag="yn",
                                              name=f"py{l}_{t}")
                                for dt in range(NDT):
                                    nc.tensor.matmul(
                                        py[:],
                                        encT[dt][:, t * 128:(t + 1) * 128],
                                        w_blk(w_next, dt),
                                        start=(dt == 0), stop=(dt == NDT - 1),
                                    )
                                nc.vector.tensor_copy(
                                    y_sb[:, 258 * t: 258 * t + 256], py[:])

                # land each allgather half, then transpose it into qT|kT
                with tc.tile_pool(name=f"qtps{l}", bufs=2, space="PSUM") as qtps:
                    for h in (1, 0):
                        sl = slice(S * h, S * h + S)
                        if skip_cc:
                            nc.vector.tensor_copy(qk_pe[:, sl], carry[:, sl])
                        else:
                            nc.sync.dma_start(qk_pe[:, sl], cc_out_h[h][0])
                            nc.sync.dma_start(carry[:, sl], cc_out_h[h][1])
                            nc.vector.tensor_tensor(
                                qk_pe[:, sl], qk_pe[:, sl], carry[:, sl],
                                mybir.AluOpType.add)
                        for half in range(2):  # 0: q, 1: k
                            for gg in range(2):
                                ptq = qtps.tile([128, 512], BF16, tag="qt",
                                                name=f"qt{l}_{h}_{half}_{gg}")
                                for m in range(4):
                                    i = 8 * h + gg * 4 + m
                                    nc.tensor.transpose(
                                        ptq[:, m * 128:(m + 1) * 128],
                                        qk_pe[:, 256 * i + 128 * half:
                                              256 * i + 128 * half + 128],
                                        identb[:],
                                    )
                                dst0 = half * S + 1024 * h + gg * 512
                                nc.vector.tensor_copy(
                                    qkT[:, dst0: dst0 + 512], ptq[:],
                                )

                # == phase S1: scoresT, exp, mask (descending t) ==
                with tc.tile_pool(name=f"exp{l}", bufs=1) as exp_pool:
                    expT = {}
                    for t in reversed(range(NT)):
                        expT[t] = exp_pool.tile(
                            [128, S - 128 * t], F32R, tag=f"e{t}",
                            name=f"expT{l}_{t}")
                    with tc.tile_pool(name=f"scps{l}", bufs=2,
                                      space="PSUM") as scps:
                        for t in reversed(range(NT)):
                            nt_cols = S - 128 * t
                            psc = scps.tile([128, S], F32, tag="sc",
                                            name=f"sc{l}_{t}")
                            kT_t = qkT[:, S + t * 128: S + (t + 1) * 128]
                            for ch in range((nt_cols + 511) // 512):
                                w = min(512, nt_cols - ch * 512)
                                nc.tensor.matmul(
                                    psc[:, ch * 512: ch * 512 + w],
                                    kT_t,
                                    qkT[:, 128 * t + ch * 512:
                                        128 * t + ch * 512 + w],
                                    start=True, stop=True,
                                )
                            nc.scalar.activation(
                                expT[t][:], psc[:, 0:nt_cols], Exp, scale=SCALE,
                            )
                            nc.vector.tensor_tensor(
                                expT[t][:, 0:128], expT[t][:, 0:128], utmask[:],
                                mybir.AluOpType.mult,
                            )

                    # == phase S2 ==
                    has_carry = l0 < L - 1
                    emit_ag = (not skip_cc) and (has_carry or reps > 1)
                    ow = DC + (258 if has_carry else 2)
                    with (
                        tc.tile_pool(name=f"ops{l}", bufs=2,
                                     space="PSUM") as ops,
                        tc.tile_pool(name=f"rc{l}", bufs=2) as rcp,
                    ):
                        for i in range(NT - 1, -1, -1):
                            pso = ops.tile([128, ow], F32, tag="o",
                                           name=f"o{l}_{i}")
                            for t in range(i + 1):
                                blk = expT[t][:, (i - t) * 128:(i - t + 1) * 128]
                                for ch in range(DC // 512):
                                    sl = slice(ch * 512, (ch + 1) * 512)
                                    nc.tensor.matmul(
                                        pso[:, sl], blk, enc[t][:, sl],
                                        start=(t == 0), stop=(t == i),
                                    )
                                if has_carry:
                                    nc.tensor.matmul(
                                        pso[:, DC:DC + 258], blk,
                                        y_sb[:, 258 * t: 258 * t + 258],
                                        start=(t == 0), stop=(t == i),
                                    )
                                else:
                                    nc.tensor.matmul(
                                        pso[:, DC:DC + 2], blk, ones[:],
                                        start=(t == 0), stop=(t == i),
                                    )
                            rec = rcp.tile([128, 1], F32, tag="r",
                                           name=f"r{l}_{i}")
                            dcol = DC + 256 if has_carry else DC
                            nc.vector.reciprocal(rec[:], pso[:, dcol:dcol + 1])
                            nc.scalar.activation(
                                enc[i][:], pso[:, 0:DC], Copy, scale=rec[:],
                            )
                            if has_carry:
                                nc.scalar.activation(
                                    carry[:, 256 * i: 256 * i + 256],
                                    pso[:, DC:DC + 256], Copy, scale=rec[:],
                                )
                            if emit_ag:
                                if i == 8:
                                    emit_ag_half(1)
                                elif i == 0:
                                    emit_ag_half(0)

            # ---- write output ----
            for u in range(NT):
                nc.sync.dma_start(out_dram[u * 128:(u + 1) * 128, :], enc[u][:])

    nc.compile()
    return nc


def _pe_table():
    pos = np.arange(S, dtype=np.float32)[:, None]
    half = np.arange(0, D, 2, dtype=np.float32)
    div = np.exp(-(np.log(np.float32(10000.0)) / np.float32(D)) * half)
    pe = np.zeros((S, D), np.float32)
    pe[:, 0::2] = np.sin(pos * div)
    pe[:, 1::2] = np.cos(pos * div)
    return pe


def _prepare_in_maps(token_ids, Wq, Wk):
    import ml_dtypes

    token_ids = np.asarray(token_ids)
    Wq = np.asarray(Wq, dtype=np.float32)
    Wk = np.asarray(Wk, dtype=np.float32)

    pe = _pe_table()
    ident = np.eye(128, dtype=np.float32)
    identb = np.eye(128, dtype=ml_dtypes.bfloat16)
    utmask = np.triu(np.ones((128, 128), np.float32))
    ones = np.ones((128, 2), np.float32)

    # stacked [wq | wk]: w_stk[l][p, 256*dt + m] = W{q,k}[l, cDC + 128dt + p, m]
    def w_stack(c):
        out = np.empty((L, 128, NDT * 256), np.float32)
        for dt in range(NDT):
            rows = slice(c * DC + dt * 128, c * DC + (dt + 1) * 128)
            out[:, :, dt * 256: dt * 256 + 128] = Wq[:, rows, :]
            out[:, :, dt * 256 + 128: dt * 256 + 256] = Wk[:, rows, :]
        return out

    in_maps = []
    for core in range(N_CORES):
        b, c = divmod(core, SPLIT)
        toks = token_ids[b % B].astype(np.float32)
        tok_col = np.ascontiguousarray(toks.reshape(NT, 128).T)  # [128, NT]
        iota_nat = np.broadcast_to(
            (np.arange(DC, dtype=np.float32) + c * DC)[None, :], (128, DC)
        ).copy()
        in_maps.append({
            "pe_nat": np.ascontiguousarray(pe[:, c * DC:(c + 1) * DC]),
            "tok_col": tok_col,
            "iota_nat": iota_nat,
            "w_stk": w_stack(c),
            "ident": ident,
            "identb": identb,
            "utmask": utmask,
            "ones": ones,
        })
    return in_maps


def kernel(token_ids, Wq, Wk, _trace=False):
    from concourse.bass_utils import run_bass_kernel_spmd

    if "nc" not in _CACHED:
        _CACHED["nc"] = _build()
    nc = _CACHED["nc"]

    if "in_maps" not in _CACHED:
        _CACHED["in_maps"] = _prepare_in_maps(token_ids, Wq, Wk)

    res = run_bass_kernel_spmd(
        nc, _CACHED["in_maps"], core_ids=list(range(N_CORES)), trace=_trace,
    )
    _CACHED["last_result"] = res

    out = np.empty((B, S, D), np.float32)
    for core in range(N_CORES):
        b, c = divmod(core, SPLIT)
        out[b][:, c * DC:(c + 1) * DC] = res.results[core]["out"]
    return out


# revision 22
# speedup vs baseline: 15052.5374x; 15052.5374x over previous
"""Trainium2 Bass kernel for nn_AttnOnlyTransformer_55929064128766.

Reference model: B=4, S=2048, D=2048 (=vocab), DQK=128, L=4 layers.
  enc0 = one_hot(token_ids, D) + sinusoidal_PE(S, D)
  per layer: q = enc@Wq; k = enc@Wk; A = softmax(mask(q k^T / sqrt(DQK)));
             enc = A @ enc
  output: enc  [B, S, D] f32

Sharding (8 cores): data-parallel over batch (4 pairs) x column-parallel
over D within each pair (Dc = 1024 columns of enc per core).  Per layer
each core computes partial q/k from its columns; a pairwise AllReduce
(groups [0,1],[2,3],[4,5],[6,7]) completes the projections; scores are
replicated within the pair; A @ enc splits cleanly by columns and the
column sharding of enc is preserved across layers.

All matmuls use float32r (f32 storage, ~1.5e-4 matmul relative error,
4x faster than plain f32 on the PE).
"""

import math

import numpy as np

B, S, D, DQK, L = 4, 2048, 2048, 128, 4
SPLIT = 2                 # cores per batch (column split factor)
DC = D // SPLIT           # columns of enc owned by one core
N_CORES = B * SPLIT
NT = S // 128             # number of 128-row tiles of the sequence (16)
NDT = DC // 128           # number of 128-col d-tiles per core (8)
SCALE = 1.0 / math.sqrt(DQK)
GROUPS = [[2 * i, 2 * i + 1] for i in range(B)]

_CACHED = {}


def _build(reps=1, skip_cc=False, no_ag=False):
    import concourse.bass as bass  # noqa: F401
    import concourse.mybir as mybir
    import concourse.tile as tile
    from concourse import bacc

    F32 = mybir.dt.float32
    F32R = mybir.dt.float32r
    BF16 = mybir.dt.bfloat16
    Exp = mybir.ActivationFunctionType.Exp
    Copy = mybir.ActivationFunctionType.Copy

    nc = bacc.Bacc("TRN2", target_bir_lowering=False, debug=False,
                   num_devices=N_CORES)

    # ---- I/O ----
    pe_nat = nc.dram_tensor("pe_nat", [S, DC], F32R, kind="ExternalInput").ap()
    tok_col = nc.dram_tensor("tok_col", [128, NT], F32, kind="ExternalInput").ap()
    iota_nat = nc.dram_tensor("iota_nat", [128, DC], F32, kind="ExternalInput").ap()
    w_stk_in = nc.dram_tensor("w_stk", [L, 128, NDT * 256], F32R,
                              kind="ExternalInput").ap()
    ident_in = nc.dram_tensor("ident", [128, 128], F32R, kind="ExternalInput").ap()
    identb_in = nc.dram_tensor("identb", [128, 128], BF16, kind="ExternalInput").ap()
    utmask_in = nc.dram_tensor("utmask", [128, 128], F32, kind="ExternalInput").ap()
    ones_in = nc.dram_tensor("ones", [128, 2], F32R, kind="ExternalInput").ap()
    out_dram = nc.dram_tensor("out", [S, DC], F32R, kind="ExternalOutput").ap()

    # two half-allgathers per layer; half h carries q/k row blocks 8h..8h+7
    cc_in_h = [nc.dram_tensor(f"cc_in{h}", [128, S], BF16, kind="Internal").ap()
               for h in range(2)]
    cc_out_h = [nc.dram_tensor(f"cc_out{h}", [SPLIT, 128, S], BF16,
                               kind="Internal").ap()
                for h in range(2)]

    with tile.TileContext(nc) as tc:
        with (
            tc.tile_pool(name="state", bufs=1) as state,
            tc.tile_pool(name="consts", bufs=1) as consts,
            tc.tile_pool(name="wpool", bufs=2) as wpool,
        ):
            enc = [state.tile([128, DC], F32R, tag=f"enc{u}", name=f"enc{u}")
                   for u in range(NT)]
            qkT = state.tile([128, 2 * S], BF16, tag="qkT", name="qkT")
            carry = state.tile([128, 2 * S], BF16, tag="carry", name="carry")
            y_sb = state.tile([128, NT * 258], F32R, tag="y", name="y")
            qk_pe = state.tile([128, 2 * S], BF16, tag="qkpe", name="qk_pe")

            ident = consts.tile([128, 128], F32R, tag="ident")
            identb = consts.tile([128, 128], BF16, tag="identb")
            utmask = consts.tile([128, 128], F32, tag="utmask")
            ones = consts.tile([128, 2], F32R, tag="ones")
            nc.sync.dma_start(ident[:], ident_in)
            nc.sync.dma_start(identb[:], identb_in)
            nc.sync.dma_start(utmask[:], utmask_in)
            nc.sync.dma_start(ones[:], ones_in)

            for t in range(NT):
                nc.vector.tensor_copy(
                    y_sb[:, 258 * t + 256: 258 * t + 258], ones[:])

            def w_blk(w_tile, dt):
                return w_tile[:, dt * 256: dt * 256 + 256]

            def emit_ag_half(h):
                # carry cols [2048h, 2048h+2048) hold q/k row blocks 8h..8h+7
                for q in range(2):
                    nc.sync.dma_start(
                        cc_in_h[h][:, q * 1024:(q + 1) * 1024],
                        carry[:, S * h + q * 1024: S * h + (q + 1) * 1024])
                if not no_ag:
                    nc.gpsimd.collective_compute(
                        "AllGather",
                        mybir.AluOpType.bypass,
                        replica_groups=GROUPS,
                        ins=[cc_in_h[h]],
                        outs=[cc_out_h[h]],
                    )

            # ---- build enc0 = PE + one_hot(tokens) ----
            with tc.tile_pool(name="tmp0", bufs=4) as tmp0:
                tok = tmp0.tile([128, NT], F32, tag="tok", name="tok")
                iota = tmp0.tile([128, DC], F32, tag="iota", name="iota")
                nc.sync.dma_start(tok[:], tok_col)
                nc.sync.dma_start(iota[:], iota_nat)
                for u in reversed(range(NT)):
                    nc.sync.dma_start(enc[u][:], pe_nat[u * 128:(u + 1) * 128, :])
                for u in reversed(range(NT)):
                    eng = nc.vector if u % 2 == 0 else nc.gpsimd
                    oh = tmp0.tile([128, DC], F32, tag=f"oh{u % 4}",
                                   name=f"oh{u}")
                    eng.tensor_scalar(
                        oh[:], iota[:], tok[:, u:u + 1], None,
                        mybir.AluOpType.is_equal,
                    )
                    eng.tensor_tensor(
                        enc[u][:], enc[u][:], oh[:], mybir.AluOpType.add,
                    )

            # ---- layers ----
            for rep in range(reps):
              for l0 in range(L):
                l = rep * L + l0

                # == phase P ==
                if l0 == 0 and rep == 0:
                    w_cur = wpool.tile([128, NDT * 256], F32R, tag="w",
                                       name=f"wcur{l}")
                    nc.sync.dma_start(w_cur[:], w_stk_in[0])
                if l0 < L - 1:
                    w_next = wpool.tile([128, NDT * 256], F32R, tag="w",
                                        name=f"wnext{l}")
                    nc.sync.dma_start(w_next[:], w_stk_in[l0 + 1])

                with tc.tile_pool(name=f"encT{l}", bufs=1) as encT_pool:
                    encT = [encT_pool.tile([128, S], F32R, tag=f"encT{j}",
                                           name=f"encT{l}_{j}")
                            for j in range(NDT)]
                    with tc.tile_pool(name=f"trps{l}", bufs=2,
                                      space="PSUM") as trps:
                        for j in range(NDT):
                            for g in reversed(range(NT // 4)):
                                umin = g * 4
                                ptr = trps.tile([128, 512], F32R, tag="tr",
                                                name=f"tr{l}_{j}_{g}")
                                for m in range(4):
                                    u = umin + m
                                    nc.tensor.transpose(
                                        ptr[:, m * 128:(m + 1) * 128],
                                        enc[u][:, j * 128:(j + 1) * 128],
                                        ident[:],
                                    )
                                nc.vector.tensor_copy(
                                    encT[j][:, umin * 128: umin * 128 + 512],
                                    ptr[:],
                                )

                    with tc.tile_pool(name=f"yps{l}", bufs=2,
                                      space="PSUM") as yps:
                        # layer 0: qk partial = enc0 @ W0 (natural), to carry
                        if l0 == 0 and rep == 0:
                            for t in reversed(range(NT)):
                                py = yps.tile([128, 256], F32, tag="y0",
                                              name=f"py0_{l}_{t}")
                                for dt in range(NDT):
                                    nc.tensor.matmul(
                                        py[:],
                                        encT[dt][:, t * 128:(t + 1) * 128],
                                        w_blk(w_cur, dt),
                                        start=(dt == 0), stop=(dt == NDT - 1),
                                    )
                                nc.vector.tensor_copy(
                                    carry[:, 256 * t: 256 * t + 256], py[:])
                                if not skip_cc:
                                    if t == 8:
                                        emit_ag_half(1)
                                    elif t == 0:
                                        emit_ag_half(0)

                        # y_{l+1} = enc_l @ W_{l+1} (overlaps the allgather)
                        if l0 < L - 1:
                            for t in range(NT):
                                py = yps.tile([128, 256], F32, tag="yn",
                                              name=f"py{l}_{t}")
                                for dt in range(NDT):
                                    nc.tensor.matmul(
                                        py[:],
                                        encT[dt][:, t * 128:(t + 1) * 128],
                                        w_blk(w_next, dt),
                                        start=(dt == 0), stop=(dt == NDT - 1),
                                    )
                                nc.vector.tensor_copy(
                                    y_sb[:, 258 * t: 258 * t + 256], py[:])

                # land each allgather half, transpose into qT|kT, and
                # immediately emit the score tiles that half unblocks
                with tc.tile_pool(name=f"exp{l}", bufs=1) as exp_pool:
                    expT = {}
                    for t in reversed(range(NT)):
                        expT[t] = exp_pool.tile(
                            [128, S - 128 * t], F32R, tag=f"e{t}",
                            name=f"expT{l}_{t}")
                    with (
                        tc.tile_pool(name=f"qtps{l}", bufs=2,
                                     space="PSUM") as qtps,
                        tc.tile_pool(name=f"scps{l}", bufs=2,
                                     space="PSUM") as scps,
                    ):
                        for h in (1, 0):
                            sl = slice(S * h, S * h + S)
                            if skip_cc:
                                nc.vector.tensor_copy(qk_pe[:, sl],
                                                      carry[:, sl])
                            else:
                                for q in range(2):
                                    qs = slice(S * h + q * 1024,
                                               S * h + (q + 1) * 1024)
                                    cs = slice(q * 1024, (q + 1) * 1024)
                                    src0 = cc_in_h[h] if no_ag else cc_out_h[h][0]
                                    src1 = cc_in_h[h] if no_ag else cc_out_h[h][1]
                                    nc.sync.dma_start(qk_pe[:, qs], src0[:, cs])
                                    nc.sync.dma_start(carry[:, qs], src1[:, cs])
                                nc.vector.tensor_tensor(
                                    qk_pe[:, sl], qk_pe[:, sl], carry[:, sl],
                                    mybir.AluOpType.add)
                            for half in range(2):  # 0: q, 1: k
                                for gg in range(2):
                                    ptq = qtps.tile([128, 512], BF16, tag="qt",
                                                    name=f"qt{l}_{h}_{half}_{gg}")
                                    for m in range(4):
                                        i = 8 * h + gg * 4 + m
                                        nc.tensor.transpose(
                                            ptq[:, m * 128:(m + 1) * 128],
                                            qk_pe[:, 256 * i + 128 * half:
                                                  256 * i + 128 * half + 128],
                                            identb[:],
                                        )
                                    dst0 = half * S + 1024 * h + gg * 512
                                    nc.vector.tensor_copy(
                                        qkT[:, dst0: dst0 + 512], ptq[:],
                                    )
                            # scores for tiles this half unblocks:
                            # h=1 -> t in 15..8 (sq range within [1024,2048))
                            # h=0 -> t in 7..0
                            for t in reversed(range(8 * h, 8 * h + 8)):
                                nt_cols = S - 128 * t
                                kT_t = qkT[:, S + t * 128: S + (t + 1) * 128]
                                for base in range(0, nt_cols, 1024):
                                    wc = min(1024, nt_cols - base)
                                    psc = scps.tile([128, 1024], F32, tag="sc",
                                                    name=f"sc{l}_{t}_{base}")
                                    for ch in range((wc + 511) // 512):
                                        w = min(512, wc - ch * 512)
                                        off = base + ch * 512
                                        nc.tensor.matmul(
                                            psc[:, ch * 512: ch * 512 + w],
                                            kT_t,
                                            qkT[:, 128 * t + off:
                                                128 * t + off + w],
                                            start=True, stop=True,
                                        )
                                    nc.scalar.activation(
                                        expT[t][:, base:base + wc],
                                        psc[:, 0:wc], Exp, scale=SCALE,
                                    )
                                    if base == 0:
                                        nc.vector.tensor_tensor(
                                            expT[t][:, 0:128],
                                            expT[t][:, 0:128],
                                            utmask[:], mybir.AluOpType.mult,
                                        )

                    # == phase S2 ==
                    has_carry = l0 < L - 1
                    emit_ag = (not skip_cc) and (has_carry or reps > 1)
                    ow = DC + (258 if has_carry else 2)
                    with (
                        tc.tile_pool(name=f"ops{l}", bufs=2,
                                     space="PSUM") as ops,
                        tc.tile_pool(name=f"rc{l}", bufs=2) as rcp,
                    ):
                        for i in range(NT - 1, -1, -1):
                            pso = ops.tile([128, ow], F32, tag="o",
                                           name=f"o{l}_{i}")
                            ts_order = (list(range(8, i + 1)) + list(range(8))
                                        if i >= 8 else list(range(i + 1)))
                            first_t, last_t = ts_order[0], ts_order[-1]
                            for t in ts_order:
                                blk = expT[t][:, (i - t) * 128:(i - t + 1) * 128]
                                for ch in range(DC // 512):
                                    sl = slice(ch * 512, (ch + 1) * 512)
                                    nc.tensor.matmul(
                                        pso[:, sl], blk, enc[t][:, sl],
                                        start=(t == first_t),
                                        stop=(t == last_t),
                                    )
                                if has_carry:
                                    nc.tensor.matmul(
                                        pso[:, DC:DC + 258], blk,
                                        y_sb[:, 258 * t: 258 * t + 258],
                                        start=(t == first_t),
                                        stop=(t == last_t),
                                    )
                                else:
                                    nc.tensor.matmul(
                                        pso[:, DC:DC + 2], blk, ones[:],
                                        start=(t == first_t),
                                        stop=(t == last_t),
                                    )
                            rec = rcp.tile([128, 1], F32, tag="r",
                                           name=f"r{l}_{i}")
                            dcol = DC + 256 if has_carry else DC
                            nc.vector.reciprocal(rec[:], pso[:, dcol:dcol + 1])
                            nc.scalar.activation(
                                enc[i][:], pso[:, 0:DC], Copy, scale=rec[:],
                            )
                            if has_carry:
                                nc.scalar.activation(
                                    carry[:, 256 * i: 256 * i + 256],
                                    pso[:, DC:DC + 256], Copy, scale=rec[:],
                                )
                            if emit_ag:
                                if i == 8:
                                    emit_ag_half(1)
                                elif i == 0:
                                    emit_ag_half(0)

            # ---- write output ----
            for u in range(NT):
                nc.sync.dma_start(out_dram[u * 128:(u + 1) * 128, :], enc[u][:])

    nc.compile()
    return nc


def _pe_table():
    pos = np.arange(S, dtype=np.float32)[:, None]
    half = np.arange(0, D, 2, dtype=np.float32)
    div = np.exp(-(np.log(np.float32(10000.0)) / np.float32(D)) * half)
    pe = np.zeros((S, D), np.float32)
    pe[:, 0::2] = np.sin(pos * div)
    pe[:, 1::2] = np.cos(pos * div)
    return pe


def _prepare_in_maps(token_ids, Wq, Wk):
    import ml_dtypes

    token_ids = np.asarray(token_ids)
    Wq = np.asarray(Wq, dtype=np.float32)
    Wk = np.asarray(Wk, dtype=np.float32)

    pe = _pe_table()
    ident = np.eye(128, dtype=np.float32)
    identb = np.eye(128, dtype=ml_dtypes.bfloat16)
    utmask = np.triu(np.ones((128, 128), np.float32))
    ones = np.ones((128, 2), np.float32)

    # stacked [wq | wk]: w_stk[l][p, 256*dt + m] = W{q,k}[l, cDC + 128dt + p, m]
    def w_stack(c):
        out = np.empty((L, 128, NDT * 256), np.float32)
        for dt in range(NDT):
            rows = slice(c * DC + dt * 128, c * DC + (dt + 1) * 128)
            out[:, :, dt * 256: dt * 256 + 128] = Wq[:, rows, :]
            out[:, :, dt * 256 + 128: dt * 256 + 256] = Wk[:, rows, :]
        return out

    in_maps = []
    for core in range(N_CORES):
        b, c = divmod(core, SPLIT)
        toks = token_ids[b % B].astype(np.float32)
        tok_col = np.ascontiguousarray(toks.reshape(NT, 128).T)  # [128, NT]
        iota_nat = np.broadcast_to(
            (np.arange(DC, dtype=np.float32) + c * DC)[None, :], (128, DC)
        ).copy()
        in_maps.append({
            "pe_nat": np.ascontiguousarray(pe[:, c * DC:(c + 1) * DC]),
            "tok_col": tok_col,
            "iota_nat": iota_nat,
            "w_stk": w_stack(c),
            "ident": ident,
            "identb": identb,
            "utmask": utmask,
            "ones": ones,
        })
    return in_maps


def kernel(token_ids, Wq, Wk, _trace=False):
    from concourse.bass_utils import run_bass_kernel_spmd

    if "nc" not in _CACHED:
        _CACHED["nc"] = _build()
    nc = _CACHED["nc"]

    import hashlib
    key = hashlib.md5(
        np.asarray(token_ids).tobytes() + np.asarray(Wq).tobytes()
        + np.asarray(Wk).tobytes()
    ).hexdigest()
    if _CACHED.get("in_key") != key:
        _CACHED["in_maps"] = _prepare_in_maps(token_ids, Wq, Wk)
        _CACHED["in_key"] = key

    res = run_bass_kernel_spmd(
        nc, _CACHED["in_maps"], core_ids=list(range(N_CORES)), trace=_trace,
    )
    _CACHED["last_result"] = res

    out = np.empty((B, S, D), np.float32)
    for core in range(N_CORES):
        b, c = divmod(core, SPLIT)
        out[b][:, c * DC:(c + 1) * DC] = res.results[core]["out"]
    return out
